# revision 41
# baseline (speedup 1.0000x reference)
"""Trainium2 Bass kernel for a dense transformer block (self-attn + cross-attn
+ MLP, each followed by a bottleneck adapter residual).

Sharding: data-parallel over batch — 8 cores, one batch element per core.
Each core runs the full block on its [1024, 768] token slab; no collectives.

Layout strategy: activations live feature-major in SBUF ([feature, token],
features on partitions) so every projection is matmul(lhsT=W[k,n], rhs=act[k,t])
with natural weight layout and zero transposes between ops. LayerNorm
mean/var are cross-partition reductions done with ones-vector matmuls.
Attention probabilities are computed transposed (keys on partitions) so
softmax normalization folds into the P@V matmul via an appended ones row,
and exp() needs no max-subtraction (scores are bounded; fp32 exp is safe).
All matmuls run in fp32r (full PE rate at moving-dim >= 256, ~1e-4 rel err).
SBUF is managed with phase-scoped tile pools (released between phases) plus
a small set of kernel-lifetime pools (residual stream, weights, constants).
"""
import numpy as np

import concourse.bass as bass
import concourse.mybir as mybir
import concourse.tile as tile
from concourse import bacc, bass_utils

P = 128
T = 1024          # tokens per core (batch element)
D = 768           # embed dim
KD = D // P       # 6 feature tiles
NH = 12
HD = 64
HD1 = HD + 1      # +1 ones row for softmax sum
NE = 257          # encoder tokens
NEP = 384         # encoder tokens padded (matmul moving dims must be even)
H4 = 4 * D        # 3072
DQ = D // 4       # 192 adapter dim
EPS = 1e-5
ATT_SCALE = 1.0 / 8.0  # 1/sqrt(64)

f32 = mybir.dt.float32
f32r = mybir.dt.float32r
f8 = mybir.dt.float8e4
AF = mybir.ActivationFunctionType
OP = mybir.AluOpType

CH_T = [(0, 512), (512, 512)]      # moving-dim chunks over tokens
CH_LN = [(i * 256, 256) for i in range(4)]  # finer LN chunks: earlier unblock
CH_NE = [(0, NEP)]
TT_T = [(i * P, P) for i in range(8)]   # token tiles
TT_NE = [(0, P), (P, P), (2 * P, 1)]      # real encoder token tiles (load)
TT_NE_KV = [(0, P), (P, P), (2 * P, P)]   # padded kv tiles (attention)


class Pool:
    """Thin wrapper: tiles named+tagged, with per-tag buf counts."""
    _uid = [0]

    def __init__(self, tc, name, space=None, side=None):
        kw = {}
        if space:
            kw["space"] = space
        if side:
            kw["side"] = side
        self.p = tc.alloc_tile_pool(name=name, bufs=1, **kw)

    def t(self, shape, dt, tag, bufs):
        Pool._uid[0] += 1
        return self.p.tile(list(shape), dt, name=f"{tag}{Pool._uid[0]}",
                           tag=tag, bufs=bufs)

    def release(self):
        self.p.release()


def emit(nc, tc, dr):
    V = nc.vector
    S = nc.scalar
    G = nc.gpsimd
    TE = nc.tensor

    def dma(out, in_):
        nc.sync.dma_start(out, in_)

    # ============ kernel-lifetime pools ============
    gc = Pool(tc, "gconst")          # constants + bias tables (~35KB)
    gr = Pool(tc, "gres")            # residual stream, 6 x [P,T] f32r (24KB)

    def gct(shape, dt, tag):
        return gc.t(shape, dt, tag, 1)

    def new_resid():
        return [gr.t([P, T], f32r, "resid", 6) for _ in range(KD)]

    # ---------------- constants ----------------
    identf = gct([P, P], f32, "identf")
    G.memset(identf[:], 0.0)
    G.affine_select(out=identf[:], in_=identf[:], compare_op=OP.not_equal,
                    fill=1.0, base=0, pattern=[[-1, P]], channel_multiplier=1)
    ident = gct([P, P], f32r, "ident")
    V.tensor_copy(ident[:], identf[:])

    onesf = gct([P, NH], f32, "onesf")
    G.memset(onesf[:], 1.0)
    ones = gct([P, 1], f32r, "ones")
    V.tensor_copy(ones[:], onesf[:, 0:1])
    onesw = gct([P, NH], f32r, "onesw")
    V.tensor_copy(onesw[:], onesf[:])

    # causal mask window [P, 896]: M[p, x] = 1.0 where p <= x - 384
    maskf = gct([P, 896], f32, "maskf")
    G.memset(maskf[:], 1.0)
    G.affine_select(out=maskf[:], in_=maskf[:], compare_op=OP.is_ge,
                    fill=0.0, base=-384, pattern=[[1, 896]],
                    channel_multiplier=-1)
    mask = gct([P, 896], f32r, "mask")
    V.tensor_copy(mask[:], maskf[:])

    epst = gct([1, 1], f32, "epst")
    G.memset(epst[:], EPS)

    zpad = gct([P, NEP - NE], f32, "zpad")
    G.memset(zpad[:], 0.0)

    maskcol = gct([P, 1], f32, "maskcol")
    G.memset(maskcol[:], 0.0)
    G.memset(maskcol[0:1, :], 1.0)

    # ============ reusable phase bodies ============
    def load_transposed(src_d, n_tok, tok_tiles, out_pool, out_tag, out_bufs,
                        n_pad=None):
        sp = Pool(tc, "ph_ldx" + out_tag)
        pp = Pool(tc, "ph_ldxp" + out_tag, space="PSUM")
        width = n_pad if n_pad else n_tok
        out = [out_pool.t([P, width], f32r, out_tag, out_bufs)
               for _ in range(KD)]
        if n_pad:
            for k in range(KD):
                V.tensor_copy(out[k][:, n_tok:width], zpad[:, 0:width - n_tok])
        for (t0, tp) in tok_tiles:
            if tp == 1:
                # single token row: DMA-scatter features across partitions
                for k in range(KD):
                    dma(out[k][:, t0:t0 + 1],
                        src_d[t0:t0 + 1, k * P:(k + 1) * P].bitcast(f32r)
                        .rearrange("a (p o) -> p (a o)", o=1))
                continue
            xt = sp.t([tp, D], f32r, "xtok", 4)
            dma(xt[:], src_d[t0:t0 + tp, :].bitcast(f32r))
            for k in range(KD):
                pst = pp.t([P, tp], f32r, "tp", 4)
                TE.transpose(pst[:], xt[:, k * P:(k + 1) * P],
                             ident[0:tp, 0:tp])
                V.tensor_copy(out[k][:, t0:t0 + tp], pst[:])
        sp.release()
        pp.release()
        return out

    def layernorm(x_tiles, n_tok, chunks, g_cols, b_cols,
                  out_pool, out_tag, out_bufs, name, out_aps=None):
        """Feature-major LN, processed independently per token-chunk so the
        first chunk's output unblocks downstream matmuls early."""
        sp = Pool(tc, "ph_ln" + name)
        pp = Pool(tc, "ph_lnp" + name, space="PSUM")
        out = out_aps if out_aps is not None else [
            out_pool.t([P, n_tok], f32r, out_tag, out_bufs)
            for _ in range(KD)]
        for (c0, cw) in chunks:
            def row(tag):
                return sp.t([1, cw], f32, tag, 2)

            stp0 = pp.t([1, cw], f32, "st0", 2)
            stp1 = pp.t([1, cw], f32, "st1", 2)
            for k in range(KD):
                TE.matmul(stp0[:], ones[:], x_tiles[k][:, c0:c0 + cw],
                          start=(k == 0), stop=(k == KD - 1))
            for k in range(KD):
                sqc = sp.t([P, cw], f32r, "sqc", 3)
                S.activation(sqc[:], x_tiles[k][:, c0:c0 + cw], AF.Square)
                TE.matmul(stp1[:], ones[:], sqc[:],
                          start=(k == 0), stop=(k == KD - 1))
            mean = row("lnm")
            V.tensor_scalar(mean[:], stp0[:], 1.0 / D, None, OP.mult)
            r = row("lnr")  # reused in place: var -> sd -> 1/sd
            V.scalar_tensor_tensor(r[:], mean[:], -1.0, mean[:],
                                   OP.mult, OP.mult)
            V.scalar_tensor_tensor(r[:], stp1[:], 1.0 / D, r[:],
                                   OP.mult, OP.add)
            S.activation(r[:], r[:], AF.Sqrt, bias=epst[:])
            V.reciprocal(r[:], r[:])
            rmn = row("lnrm")
            V.scalar_tensor_tensor(rmn[:], r[:], -1.0, mean[:],
                                   OP.mult, OP.mult)
            rb = sp.t([P, cw], f32, "lnrb", 2)
            G.partition_broadcast(rb[:], r[:])
            rmb = sp.t([P, cw], f32, "lnrmb", 2)
            G.partition_broadcast(rmb[:], rmn[:])
            for k in range(KD):
                # alternate DVE/GpSimd so the normalize chain's latency halves
                E = V if k % 2 == 0 else G
                t1 = sp.t([P, cw], f32, "lnt1", 4)
                E.tensor_mul(t1[:], x_tiles[k][:, c0:c0 + cw], rb[:])
                E.tensor_add(t1[:], t1[:], rmb[:])
                V.tensor_scalar(out[k][:, c0:c0 + cw], t1[:],
                                g_cols[:, k:k + 1], b_cols[:, k:k + 1],
                                OP.mult, OP.add)
        sp.release()
        pp.release()
        return out

    def proj_fm(wp, psp, h_tiles, w_d, col0, ncols, chunks, evict,
                w_bufs=10, ps_bufs=6):
        """Feature-major projection; evict(nt, nsz, q0, qw, psum)."""
        CW = 512
        nk = len(h_tiles)
        for c in range(0, ncols, CW):
            cw = min(CW, ncols - c)
            wts = []
            for k in range(nk):
                kp = h_tiles[k].shape[0]
                wt = wp.t([kp, cw], f32r, "w", w_bufs)
                dma(wt[:], w_d[k * P:k * P + kp, col0 + c:col0 + c + cw]
                    .bitcast(f32r))
                wts.append(wt)
            for (q0, qw) in chunks:
                o = 0
                while o < cw:
                    nsz = min(P, cw - o)
                    pq = psp.t([nsz, qw], f32, "pp", ps_bufs)
                    for k in range(nk):
                        TE.matmul(pq[:], wts[k][:, o:o + nsz],
                                  h_tiles[k][:, q0:q0 + qw],
                                  start=(k == 0), stop=(k == nk - 1))
                    evict((c + o) // P, nsz, q0, qw, pq)
                    o += nsz

    def proj_to_tiles(wp, psp, h_tiles, w_d, col0, ncols, chunks, n_tok,
                      bias_cols, bias_coloff, out_pool, out_tag, out_bufs,
                      relu=False, ps_bufs=6):
        nparts = [min(P, ncols - i * P) for i in range((ncols + P - 1) // P)]
        out = [out_pool.t([np_, n_tok], f32r, out_tag, out_bufs)
               for np_ in nparts]

        def ev(nt, nsz, q0, qw, pq):
            bsl = bias_cols[0:nsz, bias_coloff + nt:bias_coloff + nt + 1]
            if relu:
                S.activation(out[nt][:, q0:q0 + qw], pq[:], AF.Relu, bias=bsl)
            else:
                V.tensor_scalar(out[nt][:, q0:q0 + qw], pq[:], bsl, None,
                                OP.add)

        proj_fm(wp, psp, h_tiles, w_d, col0, ncols, chunks, ev,
                ps_bufs=ps_bufs)
        return out

    def proj_tm_vext(wp, psp, h_tiles, w_d, col0, tok_tiles, vbias_b,
                     out_pool, out_tag, out_bufs):
        """Token-major V projection into [tokp, NH, HD1] ext tiles."""
        CW = 512
        vext = [out_pool.t([tp, NH, HD1], f32r, out_tag, out_bufs)
                for (_, tp) in tok_tiles]
        for i, (t0, tp) in enumerate(tok_tiles):
            V.tensor_copy(vext[i][:, :, HD:HD1], onesw[0:tp, 0:NH])
        for c in range(0, D, CW):
            cw = min(CW, D - c)
            wts = []
            for k in range(KD):
                wt = wp.t([P, cw], f32r, "w", 10)
                dma(wt[:], w_d[k * P:(k + 1) * P, col0 + c:col0 + c + cw]
                    .bitcast(f32r))
                wts.append(wt)
            h0, h1 = c // HD, (c + cw) // HD
            for i, (t0, tp) in enumerate(tok_tiles):
                pq = psp.t([tp, cw], f32, "pp", 6)
                for k in range(KD):
                    TE.matmul(pq[:], h_tiles[k][:, t0:t0 + tp], wts[k][:],
                              start=(k == 0), stop=(k == KD - 1))
                V.tensor_add(vext[i][:, h0:h1, 0:HD], pq[:],
                             vbias_b[0:tp, c:c + cw])
        return vext

    def attention(qT, kT, vext, kv_tiles, chunks, causal,
                  out_pool, out_tag, name, tail_mask_ki=None):
        sp = Pool(tc, "ph_att" + name)
        pp = Pool(tc, "ph_attp" + name, space="PSUM")
        saT = [out_pool.t([P, T], f32r, out_tag, 6) for _ in range(KD)]
        for qi, (q0, qw) in enumerate(chunks):
            for h in range(NH):
                kt_i, off = (h * HD) // P, (h * HD) % P
                q_sl = qT[kt_i]
                k_sl = kT[kt_i]
                exps = {}
                for ki, (k0, kp) in enumerate(kv_tiles):
                    if causal and k0 > q0 + qw - 1:
                        continue  # block fully masked
                    spp = pp.t([kp, qw], f32, "sp", 4)
                    TE.matmul(spp[:], k_sl[off:off + HD, k0:k0 + kp],
                              q_sl[off:off + HD, q0:q0 + qw],
                              start=True, stop=True)
                    e = sp.t([kp, qw], f32r, "exp", 10)
                    S.activation(e[:], spp[:], AF.Exp, scale=ATT_SCALE)
                    if causal and k0 + kp - 1 > q0:
                        c = 384 - (k0 - q0)  # diagonal-block mask window
                        G.tensor_mul(e[:], e[:], mask[:, c:c + qw])
                    if ki == tail_mask_ki:
                        # zero padded kv rows (only partition 0 is a real key)
                        V.tensor_scalar(e[:], e[:], maskcol[0:kp, :], None,
                                        OP.mult)
                    exps[ki] = e
                valid = sorted(exps)
                pv = pp.t([HD1, qw], f32, "pv", 4)
                for j, ki in enumerate(valid):
                    TE.matmul(pv[:], vext[ki][:, h, :], exps[ki][:],
                              start=(j == 0), stop=(j == len(valid) - 1))
                rec = sp.t([1, qw], f32, "rec", 4)
                V.reciprocal(rec[:], pv[HD:HD1, :])
                rb = sp.t([HD, qw], f32, "rb", 2)
                G.partition_broadcast(rb[:], rec[:])
                V.tensor_mul(saT[kt_i][off:off + HD, q0:q0 + qw],
                             pv[0:HD, :], rb[:])
        sp.release()
        pp.release()
        return saT

    def proj_adapter_resid(src, x_old, w_d, bias_cols, name):
        """x_new = x_old + adapter(src @ w_d + bias). Fuses the output
        projection (attn_proj / cross_proj; identity for MLP) w/ the adapter."""
        sp = Pool(tc, "ph_pa" + name)
        pp = Pool(tc, "ph_pap" + name, space="PSUM")
        if w_d is not None:
            proj = proj_to_tiles(sp, pp, src, w_d, 0, D, CH_T, T,
                                 bias_cols, 0, sp, "prj", 6, ps_bufs=4)
        else:
            proj = src
        a1 = [sp.t([P, T], f32r, "a1", 2), sp.t([DQ - P, T], f32r, "a1", 2)]
        for (q0, qw) in CH_T:
            for nt, (no, nsz) in enumerate([(0, P), (P, DQ - P)]):
                pq = pp.t([nsz, qw], f32, "a1ps", 2)
                for k in range(KD):
                    TE.matmul(pq[:], ad1w[k][:, no:no + nsz],
                              proj[k][:, q0:q0 + qw],
                              start=(k == 0), stop=(k == KD - 1))
                S.activation(a1[nt][:, q0:q0 + qw], pq[:], AF.Relu,
                             bias=b_ad1[0:nsz, nt:nt + 1])
            # residual updated in place: x_old's only other reader (its LN)
            # is upstream of this adapter chain, so the WAR is ordered
            for nt in range(KD):
                pq = pp.t([P, qw], f32, "a2ps", 2)
                for k in range(2):
                    TE.matmul(pq[:], ad2w[k][:, nt * P:(nt + 1) * P],
                              a1[k][:, q0:q0 + qw],
                              start=(k == 0), stop=(k == 1))
                V.scalar_tensor_tensor(x_old[nt][:, q0:q0 + qw], pq[:],
                                       b_ad2[:, nt:nt + 1],
                                       x_old[nt][:, q0:q0 + qw],
                                       OP.add, OP.add)
        sp.release()
        pp.release()
        return x_old

    # ================= the block =================
    # load x first so its DMA isn't queued behind the weight/bias traffic
    x0 = load_transposed(dr["x"], T, TT_T, gr, "resid", 6)

    # ---------------- small-vector loads ----------------
    def load_cols(name, n):
        """[n] dram vector -> [P, ceil(n/P)] tile; col a = vals[a*P:(a+1)*P]."""
        cols = (n + P - 1) // P
        t = gct([P, cols], f32, "b_" + name)
        full = (n // P) * P
        if full:
            dma(t[:, 0:n // P], dr[name][0:full].rearrange("(a p) -> p a", p=P))
        if n % P:
            dma(t[0:n % P, cols - 1:cols],
                dr[name][full:n].rearrange("(p o) -> p o", o=1))
        return t

    b_ln1g = load_cols("ln1_g", D)
    b_ln1b = load_cols("ln1_b", D)
    b_ln2g = load_cols("ln2_g", D)
    b_ln2b = load_cols("ln2_b", D)
    b_ln3g = load_cols("ln3_g", D)
    b_ln3b = load_cols("ln3_b", D)
    b_attn = load_cols("attn_bias", 3 * D)   # q cols 0..5, k 6..11, v 12..17
    b_aproj = load_cols("attn_proj_b", D)
    b_img = load_cols("img_b", 2 * D)        # k2 cols 0..5, v2 6..11
    b_cap = load_cols("cap_b", D)
    b_cproj = load_cols("cross_proj_b", D)
    b_ad1 = load_cols("ad1_b", DQ)
    b_ad2 = load_cols("ad2_b", D)
    b_fc = load_cols("fc_b", H4)
    b_mproj = load_cols("mproj_b", D)

    def bias_row_bcast(name, off, n):
        row = gct([1, n], f32, "vr_" + name)
        dma(row[:], dr[name][off:off + n].rearrange("(o a) -> o a", o=1))
        b = gct([P, n], f32, "vb_" + name)
        G.partition_broadcast(b[:], row[:])
        return b

    vb_attn = bias_row_bcast("attn_bias", 2 * D, D)
    vb_img = bias_row_bcast("img_b", D, D)

    # adapter weights, resident for all three adapter calls (~10.5KB)
    ad1w = []
    for k in range(KD):
        wt = gct([P, DQ], f32r, f"ad1w{k}")
        dma(wt[:], dr["ad1_w"][k * P:(k + 1) * P, :].bitcast(f32r))
        ad1w.append(wt)
    ad2w = []
    for k, kp in enumerate([P, DQ - P]):
        wt = gct([kp, D], f32r, f"ad2w{k}")
        dma(wt[:], dr["ad2_w"][k * P:k * P + kp, :].bitcast(f32r))
        ad2w.append(wt)


    # ---- encoder branch (independent of the decoder stream) ----
    s_kv = Pool(tc, "st_kv", side="right")
    enc = Pool(tc, "ph_enc")
    xe = load_transposed(dr["x_enc"], NE, TT_NE, enc, "xe", 6, n_pad=NEP)
    he = layernorm(xe, NEP, CH_NE, b_ln3g, b_ln3b, enc, "he", 6, "3")
    encp = Pool(tc, "ph_encp", space="PSUM")
    k2T = proj_to_tiles(enc, encp, he, dr["img_w"], 0, D, CH_NE, NEP,
                        b_img, 0, s_kv, "k2", 6)
    v2ext = proj_tm_vext(enc, encp, he, dr["img_w"], D, TT_NE_KV, vb_img,
                         s_kv, "v2ext", 3)
    encp.release()
    enc.release()

    # ---- self-attention ----
    # overlapping stage lifetimes alternate allocation sides (LIFO per side)
    s_qk = Pool(tc, "st_qk", side="right")
    s_h = Pool(tc, "st_h")
    h1 = layernorm(x0, T, CH_T, b_ln1g, b_ln1b, s_h, "h", 6, "1")
    ph = Pool(tc, "ph_qkv")
    php = Pool(tc, "ph_qkvp", space="PSUM")
    qT = proj_to_tiles(ph, php, h1, dr["attn_w"], 0, D, CH_T, T,
                       b_attn, 0, s_qk, "qk", 12)
    kT = proj_to_tiles(ph, php, h1, dr["attn_w"], D, D, CH_T, T,
                       b_attn, KD, s_qk, "qk", 12)
    vext = proj_tm_vext(ph, php, h1, dr["attn_w"], 2 * D, TT_T, vb_attn,
                        s_qk, "vext", 8)
    php.release()
    ph.release()
    s_h.release()
    s_sa = Pool(tc, "st_sa")
    saT = attention(qT, kT, vext, TT_T, CH_T, True, s_sa, "sa", "1")
    s_qk.release()
    x1 = proj_adapter_resid(saT, x0, dr["attn_proj_w"], b_aproj, "1")
    s_sa.release()

    # ---- cross-attention ----
    s_q2 = Pool(tc, "st_q2", side="right")
    s_h2 = Pool(tc, "st_h2")
    hq = layernorm(x1, T, CH_T, b_ln1g, b_ln1b, s_h2, "h", 6, "q")
    ph2 = Pool(tc, "ph_q2")
    ph2p = Pool(tc, "ph_q2p", space="PSUM")
    q2T = proj_to_tiles(ph2, ph2p, hq, dr["cap_w"], 0, D, CH_T, T,
                        b_cap, 0, s_q2, "q2", 6)
    ph2p.release()
    ph2.release()
    s_h2.release()
    s_ca = Pool(tc, "st_ca")
    caT = attention(q2T, k2T, v2ext, TT_NE_KV, CH_T, False, s_ca, "ca", "2",
                    tail_mask_ki=2)
    s_q2.release()
    s_kv.release()
    x2 = proj_adapter_resid(caT, x1, dr["cross_proj_w"], b_cproj, "2")
    s_ca.release()

    # ---- MLP (fp8 + DoubleRow; H4 contracted in quarters) ----
    # The MLP's output only reaches the residual through the 0.02-scale
    # adapter bottleneck, so fp8 quantization here adds ~3e-4 output error
    # while halving PE time for the two big GEMMs.
    s_m = Pool(tc, "st_m", side="right")
    s_hm = Pool(tc, "st_hm")
    # hm grouped in one tile so DoubleRow can pair adjacent feature tiles
    hm_big = s_hm.t([P, KD, T], f8, "hmbig", 1)
    hm = [hm_big[:, k, :] for k in range(KD)]
    layernorm(x2, T, CH_T, b_ln2g, b_ln2b, s_hm, "h", 6, "2", out_aps=hm)
    m = [s_m.t([P, T], f32r, "m", 6) for _ in range(KD)]
    mp = Pool(tc, "ph_mlp")
    mpp = Pool(tc, "ph_mlpp", space="PSUM")
    macc = [mp.t([P, T], f32, "macc", 6) for _ in range(KD)]
    NQ = 4                       # quarters
    QH = H4 // NQ                # 768 cols per quarter
    NKH = QH // P                # 6 contraction tiles per quarter
    def fc_quarter(quarter):
        hc0 = quarter * QH
        gt_big = mp.t([P, NKH, T], f8, "gtbig", 2)
        CW = 512
        for c in range(0, QH, CW):
            cw = min(CW, QH - c)
            wts = []
            for j in range(KD // 2):
                wtmp = mp.t([P, 2, cw], f32, "fcwf", 3)
                dma(wtmp[:], dr["fc_w"][2 * j * P:(2 * j + 2) * P,
                                        hc0 + c:hc0 + c + cw]
                    .rearrange("(j p) m -> p j m", p=P))
                wt = mp.t([P, 2, cw], f8, "fcw", 3)
                V.tensor_copy(wt[:], wtmp[:])
                wts.append(wt)
            for (q0, qw) in CH_T:
                o = 0
                while o < cw:
                    nsz = min(P, cw - o)
                    nt = (c + o) // P
                    pq = mpp.t([nsz, qw], f32, "fcps", 4)
                    for j in range(KD // 2):
                        TE.matmul(pq[:], wts[j][:, :, o:o + nsz],
                                  hm_big[:, 2 * j:2 * j + 2, q0:q0 + qw],
                                  start=(j == 0), stop=(j == KD // 2 - 1),
                                  perf_mode=mybir.MatmulPerfMode.DoubleRow)
                    bsl = b_fc[:, (hc0 + nt * P) // P:(hc0 + nt * P) // P + 1]
                    S.activation(gt_big[:, nt, q0:q0 + qw], pq[:],
                                 AF.Gelu_apprx_tanh, bias=bsl)
                    o += nsz
        mpw = []
        for j in range(NKH // 2):
            wtmp = mp.t([P, 2, D], f32, "mpwf", 3)
            dma(wtmp[:], dr["mproj_w"][hc0 + 2 * j * P:hc0 + (2 * j + 2) * P, :]
                .rearrange("(j p) m -> p j m", p=P))
            wt = mp.t([P, 2, D], f8, "mpw", 6)
            V.tensor_copy(wt[:], wtmp[:])
            mpw.append(wt)
        return gt_big, mpw

    def mproj_quarter(quarter, gt_big, mpw):
        for (q0, qw) in CH_T:
            for nt in range(KD):
                pq = mpp.t([P, qw], f32, "mpps", 4)
                for j in range(NKH // 2):
                    TE.matmul(pq[:], mpw[j][:, :, nt * P:(nt + 1) * P],
                              gt_big[:, 2 * j:2 * j + 2, q0:q0 + qw],
                              start=(j == 0), stop=(j == NKH // 2 - 1),
                              perf_mode=mybir.MatmulPerfMode.DoubleRow)
                if quarter == 0:
                    V.tensor_copy(macc[nt][:, q0:q0 + qw], pq[:])
                elif quarter < NQ - 1:
                    V.tensor_add(macc[nt][:, q0:q0 + qw], pq[:],
                                 macc[nt][:, q0:q0 + qw])
                else:
                    V.scalar_tensor_tensor(m[nt][:, q0:q0 + qw], pq[:],
                                           b_mproj[:, nt:nt + 1],
                                           macc[nt][:, q0:q0 + qw],
                                           OP.add, OP.add)

    # software pipeline: quarter q's mproj is emitted after quarter q+1's fc
    prev = None
    for quarter in range(NQ):
        cur = fc_quarter(quarter)
        if prev is not None:
            mproj_quarter(quarter - 1, *prev)
        prev = cur
    mproj_quarter(NQ - 1, *prev)
    mpp.release()
    mp.release()
    s_hm.release()
    x3 = proj_adapter_resid(m, x2, None, None, "3")
    s_m.release()

    # ---- transpose back and store ----
    op_ = Pool(tc, "ph_out")
    opp = Pool(tc, "ph_outp", space="PSUM")
    for (t0, tp) in TT_T:
        ot = op_.t([P, D], f32, "ot", 4)
        for k in range(KD):
            pst = opp.t([P, P], f32r, "otps", 4)
            TE.transpose(pst[:], x3[k][:, t0:t0 + tp], ident[:])
            V.tensor_copy(ot[:, k * P:(k + 1) * P], pst[:])
        dma(dr["out"][t0:t0 + tp, :], ot[:])
    opp.release()
    op_.release()
    gr.release()
    gc.release()


def build():
    nc = bacc.Bacc("TRN2", target_bir_lowering=False, debug=False,
                   num_devices=8)
    dr = {}
    dr["x"] = nc.dram_tensor("x", (T, D), f32, kind="ExternalInput")
    dr["x_enc"] = nc.dram_tensor("x_enc", (NE, D), f32, kind="ExternalInput")
    for nm, shp in [
        ("ln1_g", (D,)), ("ln1_b", (D,)), ("ln2_g", (D,)), ("ln2_b", (D,)),
        ("ln3_g", (D,)), ("ln3_b", (D,)),
        ("attn_w", (D, 3 * D)), ("attn_bias", (3 * D,)),
        ("attn_proj_w", (D, D)), ("attn_proj_b", (D,)),
        ("img_w", (D, 2 * D)), ("img_b", (2 * D,)),
        ("cap_w", (D, D)), ("cap_b", (D,)),
        ("cross_proj_w", (D, D)), ("cross_proj_b", (D,)),
        ("ad1_w", (D, DQ)), ("ad1_b", (DQ,)),
        ("ad2_w", (DQ, D)), ("ad2_b", (D,)),
        ("fc_w", (D, H4)), ("fc_b", (H4,)),
        ("mproj_w", (H4, D)), ("mproj_b", (D,)),
    ]:
        dr[nm] = nc.dram_tensor(nm, shp, f32, kind="ExternalInput")
    dr["out"] = nc.dram_tensor("out", (T, D), f32, kind="ExternalOutput")

    with tile.TileContext(nc) as tc:
        emit(nc, tc, dr)
    nc.compile()
    return nc


_NC = None


def kernel(**inputs):
    global _NC
    if _NC is None:
        _NC = build()
    nc = _NC
    B = inputs["x"].shape[0]
    in_maps = []
    for b in range(B):
        m = {}
        for n, v in inputs.items():
            v = np.asarray(v, dtype=np.float32)
            if n in ("x", "x_enc"):
                m[n] = np.ascontiguousarray(v[b])
            else:
                m[n] = np.ascontiguousarray(v)
        in_maps.append(m)
    res = bass_utils.run_bass_kernel_spmd(nc, in_maps, list(range(B)))
    x_out = np.stack([res.results[b]["out"] for b in range(B)], axis=0)
    # the block returns (x, x_enc); x_enc passes through unchanged
    x_enc = np.asarray(inputs["x_enc"], dtype=np.float32)
    return x_out, x_enc


# revision 42
# speedup vs baseline: 1.0025x; 1.0025x over previous
"""Trainium2 Bass kernel for a dense transformer block (self-attn + cross-attn
+ MLP, each followed by a bottleneck adapter residual).

Sharding: data-parallel over batch — 8 cores, one batch element per core.
Each core runs the full block on its [1024, 768] token slab; no collectives.

Layout strategy: activations live feature-major in SBUF ([feature, token],
features on partitions) so every projection is matmul(lhsT=W[k,n], rhs=act[k,t])
with natural weight layout and zero transposes between ops. LayerNorm
mean/var are cross-partition reductions done with ones-vector matmuls.
Attention probabilities are computed transposed (keys on partitions) so
softmax normalization folds into the P@V matmul via an appended ones row,
and exp() needs no max-subtraction (scores are bounded; fp32 exp is safe).
All matmuls run in fp32r (full PE rate at moving-dim >= 256, ~1e-4 rel err).
SBUF is managed with phase-scoped tile pools (released between phases) plus
a small set of kernel-lifetime pools (residual stream, weights, constants).
"""
import numpy as np

import concourse.bass as bass
import concourse.mybir as mybir
import concourse.tile as tile
from concourse import bacc, bass_utils

P = 128
T = 1024          # tokens per core (batch element)
D = 768           # embed dim
KD = D // P       # 6 feature tiles
NH = 12
HD = 64
HD1 = HD + 1      # +1 ones row for softmax sum
NE = 257          # encoder tokens
NEP = 384         # encoder tokens padded (matmul moving dims must be even)
H4 = 4 * D        # 3072
DQ = D // 4       # 192 adapter dim
EPS = 1e-5
ATT_SCALE = 1.0 / 8.0  # 1/sqrt(64)

f32 = mybir.dt.float32
f32r = mybir.dt.float32r
f8 = mybir.dt.float8e4
AF = mybir.ActivationFunctionType
OP = mybir.AluOpType

CH_T = [(0, 512), (512, 512)]      # moving-dim chunks over tokens
CH_LN = [(i * 256, 256) for i in range(4)]  # finer LN chunks: earlier unblock
CH_NE = [(0, NEP)]
TT_T = [(i * P, P) for i in range(8)]   # token tiles
TT_NE = [(0, P), (P, P), (2 * P, 1)]      # real encoder token tiles (load)
TT_NE_KV = [(0, P), (P, P), (2 * P, P)]   # padded kv tiles (attention)


class Pool:
    """Thin wrapper: tiles named+tagged, with per-tag buf counts."""
    _uid = [0]

    def __init__(self, tc, name, space=None, side=None):
        kw = {}
        if space:
            kw["space"] = space
        if side:
            kw["side"] = side
        self.p = tc.alloc_tile_pool(name=name, bufs=1, **kw)

    def t(self, shape, dt, tag, bufs):
        Pool._uid[0] += 1
        return self.p.tile(list(shape), dt, name=f"{tag}{Pool._uid[0]}",
                           tag=tag, bufs=bufs)

    def release(self):
        self.p.release()


def emit(nc, tc, dr):
    V = nc.vector
    S = nc.scalar
    G = nc.gpsimd
    TE = nc.tensor

    def dma(out, in_):
        nc.sync.dma_start(out, in_)

    # ============ kernel-lifetime pools ============
    gc = Pool(tc, "gconst")          # constants + bias tables (~35KB)
    gr = Pool(tc, "gres")            # residual stream, 6 x [P,T] f32r (24KB)

    def gct(shape, dt, tag):
        return gc.t(shape, dt, tag, 1)

    def new_resid():
        return [gr.t([P, T], f32r, "resid", 6) for _ in range(KD)]

    # ---------------- constants ----------------
    identf = gct([P, P], f32, "identf")
    G.memset(identf[:], 0.0)
    G.affine_select(out=identf[:], in_=identf[:], compare_op=OP.not_equal,
                    fill=1.0, base=0, pattern=[[-1, P]], channel_multiplier=1)
    ident = gct([P, P], f32r, "ident")
    V.tensor_copy(ident[:], identf[:])

    onesf = gct([P, NH], f32, "onesf")
    G.memset(onesf[:], 1.0)
    ones = gct([P, 1], f32r, "ones")
    V.tensor_copy(ones[:], onesf[:, 0:1])
    onesw = gct([P, NH], f32r, "onesw")
    V.tensor_copy(onesw[:], onesf[:])

    # causal mask window [P, 896]: M[p, x] = 1.0 where p <= x - 384
    maskf = gct([P, 896], f32, "maskf")
    G.memset(maskf[:], 1.0)
    G.affine_select(out=maskf[:], in_=maskf[:], compare_op=OP.is_ge,
                    fill=0.0, base=-384, pattern=[[1, 896]],
                    channel_multiplier=-1)
    mask = gct([P, 896], f32r, "mask")
    V.tensor_copy(mask[:], maskf[:])

    epst = gct([1, 1], f32, "epst")
    G.memset(epst[:], EPS)

    zpad = gct([P, NEP - NE], f32, "zpad")
    G.memset(zpad[:], 0.0)

    maskcol = gct([P, 1], f32, "maskcol")
    G.memset(maskcol[:], 0.0)
    G.memset(maskcol[0:1, :], 1.0)

    # ============ reusable phase bodies ============
    def load_transposed(src_d, n_tok, tok_tiles, out_pool, out_tag, out_bufs,
                        n_pad=None):
        sp = Pool(tc, "ph_ldx" + out_tag)
        pp = Pool(tc, "ph_ldxp" + out_tag, space="PSUM")
        width = n_pad if n_pad else n_tok
        out = [out_pool.t([P, width], f32r, out_tag, out_bufs)
               for _ in range(KD)]
        if n_pad:
            for k in range(KD):
                V.tensor_copy(out[k][:, n_tok:width], zpad[:, 0:width - n_tok])
        for (t0, tp) in tok_tiles:
            if tp == 1:
                # single token row: DMA-scatter features across partitions
                for k in range(KD):
                    dma(out[k][:, t0:t0 + 1],
                        src_d[t0:t0 + 1, k * P:(k + 1) * P].bitcast(f32r)
                        .rearrange("a (p o) -> p (a o)", o=1))
                continue
            xt = sp.t([tp, D], f32r, "xtok", 4)
            dma(xt[:], src_d[t0:t0 + tp, :].bitcast(f32r))
            for k in range(KD):
                pst = pp.t([P, tp], f32r, "tp", 4)
                TE.transpose(pst[:], xt[:, k * P:(k + 1) * P],
                             ident[0:tp, 0:tp])
                V.tensor_copy(out[k][:, t0:t0 + tp], pst[:])
        sp.release()
        pp.release()
        return out

    def layernorm(x_tiles, n_tok, chunks, g_cols, b_cols,
                  out_pool, out_tag, out_bufs, name, out_aps=None):
        """Feature-major LN, processed independently per token-chunk so the
        first chunk's output unblocks downstream matmuls early."""
        sp = Pool(tc, "ph_ln" + name)
        pp = Pool(tc, "ph_lnp" + name, space="PSUM")
        out = out_aps if out_aps is not None else [
            out_pool.t([P, n_tok], f32r, out_tag, out_bufs)
            for _ in range(KD)]
        for (c0, cw) in chunks:
            def row(tag):
                return sp.t([1, cw], f32, tag, 2)

            stp0 = pp.t([1, cw], f32, "st0", 2)
            stp1 = pp.t([1, cw], f32, "st1", 2)
            for k in range(KD):
                TE.matmul(stp0[:], ones[:], x_tiles[k][:, c0:c0 + cw],
                          start=(k == 0), stop=(k == KD - 1))
            for k in range(KD):
                sqc = sp.t([P, cw], f32r, "sqc", 3)
                S.activation(sqc[:], x_tiles[k][:, c0:c0 + cw], AF.Square)
                TE.matmul(stp1[:], ones[:], sqc[:],
                          start=(k == 0), stop=(k == KD - 1))
            mean = row("lnm")
            V.tensor_scalar(mean[:], stp0[:], 1.0 / D, None, OP.mult)
            r = row("lnr")  # reused in place: var -> sd -> 1/sd
            V.scalar_tensor_tensor(r[:], mean[:], -1.0, mean[:],
                                   OP.mult, OP.mult)
            V.scalar_tensor_tensor(r[:], stp1[:], 1.0 / D, r[:],
                                   OP.mult, OP.add)
            S.activation(r[:], r[:], AF.Sqrt, bias=epst[:])
            V.reciprocal(r[:], r[:])
            rmn = row("lnrm")
            V.scalar_tensor_tensor(rmn[:], r[:], -1.0, mean[:],
                                   OP.mult, OP.mult)
            rb = sp.t([P, cw], f32, "lnrb", 2)
            G.partition_broadcast(rb[:], r[:])
            rmb = sp.t([P, cw], f32, "lnrmb", 2)
            G.partition_broadcast(rmb[:], rmn[:])
            for k in range(KD):
                # alternate DVE/GpSimd so the normalize chain's latency halves
                E = V if k % 2 == 0 else G
                t1 = sp.t([P, cw], f32, "lnt1", 4)
                E.tensor_mul(t1[:], x_tiles[k][:, c0:c0 + cw], rb[:])
                E.tensor_add(t1[:], t1[:], rmb[:])
                V.tensor_scalar(out[k][:, c0:c0 + cw], t1[:],
                                g_cols[:, k:k + 1], b_cols[:, k:k + 1],
                                OP.mult, OP.add)
        sp.release()
        pp.release()
        return out

    def proj_fm(wp, psp, h_tiles, w_d, col0, ncols, chunks, evict,
                w_bufs=10, ps_bufs=6):
        """Feature-major projection; evict(nt, nsz, q0, qw, psum)."""
        CW = 512
        nk = len(h_tiles)
        for c in range(0, ncols, CW):
            cw = min(CW, ncols - c)
            wts = []
            for k in range(nk):
                kp = h_tiles[k].shape[0]
                wt = wp.t([kp, cw], f32r, "w", w_bufs)
                dma(wt[:], w_d[k * P:k * P + kp, col0 + c:col0 + c + cw]
                    .bitcast(f32r))
                wts.append(wt)
            for (q0, qw) in chunks:
                o = 0
                while o < cw:
                    nsz = min(P, cw - o)
                    pq = psp.t([nsz, qw], f32, "pp", ps_bufs)
                    for k in range(nk):
                        TE.matmul(pq[:], wts[k][:, o:o + nsz],
                                  h_tiles[k][:, q0:q0 + qw],
                                  start=(k == 0), stop=(k == nk - 1))
                    evict((c + o) // P, nsz, q0, qw, pq)
                    o += nsz

    def proj_to_tiles(wp, psp, h_tiles, w_d, col0, ncols, chunks, n_tok,
                      bias_cols, bias_coloff, out_pool, out_tag, out_bufs,
                      relu=False, ps_bufs=6):
        nparts = [min(P, ncols - i * P) for i in range((ncols + P - 1) // P)]
        out = [out_pool.t([np_, n_tok], f32r, out_tag, out_bufs)
               for np_ in nparts]

        def ev(nt, nsz, q0, qw, pq):
            bsl = bias_cols[0:nsz, bias_coloff + nt:bias_coloff + nt + 1]
            if relu:
                S.activation(out[nt][:, q0:q0 + qw], pq[:], AF.Relu, bias=bsl)
            else:
                V.tensor_scalar(out[nt][:, q0:q0 + qw], pq[:], bsl, None,
                                OP.add)

        proj_fm(wp, psp, h_tiles, w_d, col0, ncols, chunks, ev,
                ps_bufs=ps_bufs)
        return out

    def proj_tm_vext(wp, psp, h_tiles, w_d, col0, tok_tiles, vbias_b,
                     out_pool, out_tag, out_bufs):
        """Token-major V projection into [tokp, NH, HD1] ext tiles."""
        CW = 512
        vext = [out_pool.t([tp, NH, HD1], f32r, out_tag, out_bufs)
                for (_, tp) in tok_tiles]
        for i, (t0, tp) in enumerate(tok_tiles):
            V.tensor_copy(vext[i][:, :, HD:HD1], onesw[0:tp, 0:NH])
        for c in range(0, D, CW):
            cw = min(CW, D - c)
            wts = []
            for k in range(KD):
                wt = wp.t([P, cw], f32r, "w", 10)
                dma(wt[:], w_d[k * P:(k + 1) * P, col0 + c:col0 + c + cw]
                    .bitcast(f32r))
                wts.append(wt)
            h0, h1 = c // HD, (c + cw) // HD
            for i, (t0, tp) in enumerate(tok_tiles):
                pq = psp.t([tp, cw], f32, "pp", 6)
                for k in range(KD):
                    TE.matmul(pq[:], h_tiles[k][:, t0:t0 + tp], wts[k][:],
                              start=(k == 0), stop=(k == KD - 1))
                V.tensor_add(vext[i][:, h0:h1, 0:HD], pq[:],
                             vbias_b[0:tp, c:c + cw])
        return vext

    def attention(qT, kT, vext, kv_tiles, chunks, causal,
                  out_pool, out_tag, name, tail_mask_ki=None):
        sp = Pool(tc, "ph_att" + name)
        pp = Pool(tc, "ph_attp" + name, space="PSUM")
        saT = [out_pool.t([P, T], f32r, out_tag, 6) for _ in range(KD)]
        for qi, (q0, qw) in enumerate(chunks):
            for h in range(NH):
                kt_i, off = (h * HD) // P, (h * HD) % P
                q_sl = qT[kt_i]
                k_sl = kT[kt_i]
                exps = {}
                for ki, (k0, kp) in enumerate(kv_tiles):
                    if causal and k0 > q0 + qw - 1:
                        continue  # block fully masked
                    spp = pp.t([kp, qw], f32, "sp", 4)
                    TE.matmul(spp[:], k_sl[off:off + HD, k0:k0 + kp],
                              q_sl[off:off + HD, q0:q0 + qw],
                              start=True, stop=True)
                    e = sp.t([kp, qw], f32r, "exp", 10)
                    S.activation(e[:], spp[:], AF.Exp, scale=ATT_SCALE)
                    if causal and k0 + kp - 1 > q0:
                        c = 384 - (k0 - q0)  # diagonal-block mask window
                        G.tensor_mul(e[:], e[:], mask[:, c:c + qw])
                    if ki == tail_mask_ki:
                        # zero padded kv rows (only partition 0 is a real key)
                        V.tensor_scalar(e[:], e[:], maskcol[0:kp, :], None,
                                        OP.mult)
                    exps[ki] = e
                valid = sorted(exps)
                pv = pp.t([HD1, qw], f32, "pv", 4)
                for j, ki in enumerate(valid):
                    TE.matmul(pv[:], vext[ki][:, h, :], exps[ki][:],
                              start=(j == 0), stop=(j == len(valid) - 1))
                rec = sp.t([1, qw], f32, "rec", 4)
                V.reciprocal(rec[:], pv[HD:HD1, :])
                rb = sp.t([HD, qw], f32, "rb", 2)
                G.partition_broadcast(rb[:], rec[:])
                V.tensor_mul(saT[kt_i][off:off + HD, q0:q0 + qw],
                             pv[0:HD, :], rb[:])
        sp.release()
        pp.release()
        return saT

    def proj_adapter_resid(src, x_old, w_d, bias_cols, name, post_qc=None):
        """x_new = x_old + adapter(src @ w_d + bias). Fuses the output
        projection (attn_proj / cross_proj; identity for MLP) w/ the adapter."""
        sp = Pool(tc, "ph_pa" + name)
        pp = Pool(tc, "ph_pap" + name, space="PSUM")
        if w_d is not None:
            proj = proj_to_tiles(sp, pp, src, w_d, 0, D, CH_T, T,
                                 bias_cols, 0, sp, "prj", 6, ps_bufs=4)
        else:
            proj = src
        a1 = [sp.t([P, T], f32r, "a1", 2), sp.t([DQ - P, T], f32r, "a1", 2)]
        for (q0, qw) in CH_T:
            for nt, (no, nsz) in enumerate([(0, P), (P, DQ - P)]):
                pq = pp.t([nsz, qw], f32, "a1ps", 2)
                for k in range(KD):
                    TE.matmul(pq[:], ad1w[k][:, no:no + nsz],
                              proj[k][:, q0:q0 + qw],
                              start=(k == 0), stop=(k == KD - 1))
                S.activation(a1[nt][:, q0:q0 + qw], pq[:], AF.Relu,
                             bias=b_ad1[0:nsz, nt:nt + 1])
            # residual updated in place: x_old's only other reader (its LN)
            # is upstream of this adapter chain, so the WAR is ordered
            for nt in range(KD):
                pq = pp.t([P, qw], f32, "a2ps", 2)
                for k in range(2):
                    TE.matmul(pq[:], ad2w[k][:, nt * P:(nt + 1) * P],
                              a1[k][:, q0:q0 + qw],
                              start=(k == 0), stop=(k == 1))
                V.scalar_tensor_tensor(x_old[nt][:, q0:q0 + qw], pq[:],
                                       b_ad2[:, nt:nt + 1],
                                       x_old[nt][:, q0:q0 + qw],
                                       OP.add, OP.add)
            if post_qc is not None:
                post_qc(q0, qw)
        sp.release()
        pp.release()
        return x_old

    # ================= the block =================
    # load x first so its DMA isn't queued behind the weight/bias traffic
    x0 = load_transposed(dr["x"], T, TT_T, gr, "resid", 6)

    # ---------------- small-vector loads ----------------
    def load_cols(name, n):
        """[n] dram vector -> [P, ceil(n/P)] tile; col a = vals[a*P:(a+1)*P]."""
        cols = (n + P - 1) // P
        t = gct([P, cols], f32, "b_" + name)
        full = (n // P) * P
        if full:
            dma(t[:, 0:n // P], dr[name][0:full].rearrange("(a p) -> p a", p=P))
        if n % P:
            dma(t[0:n % P, cols - 1:cols],
                dr[name][full:n].rearrange("(p o) -> p o", o=1))
        return t

    b_ln1g = load_cols("ln1_g", D)
    b_ln1b = load_cols("ln1_b", D)
    b_ln2g = load_cols("ln2_g", D)
    b_ln2b = load_cols("ln2_b", D)
    b_ln3g = load_cols("ln3_g", D)
    b_ln3b = load_cols("ln3_b", D)
    b_attn = load_cols("attn_bias", 3 * D)   # q cols 0..5, k 6..11, v 12..17
    b_aproj = load_cols("attn_proj_b", D)
    b_img = load_cols("img_b", 2 * D)        # k2 cols 0..5, v2 6..11
    b_cap = load_cols("cap_b", D)
    b_cproj = load_cols("cross_proj_b", D)
    b_ad1 = load_cols("ad1_b", DQ)
    b_ad2 = load_cols("ad2_b", D)
    b_fc = load_cols("fc_b", H4)
    b_mproj = load_cols("mproj_b", D)

    def bias_row_bcast(name, off, n):
        row = gct([1, n], f32, "vr_" + name)
        dma(row[:], dr[name][off:off + n].rearrange("(o a) -> o a", o=1))
        b = gct([P, n], f32, "vb_" + name)
        G.partition_broadcast(b[:], row[:])
        return b

    vb_attn = bias_row_bcast("attn_bias", 2 * D, D)
    vb_img = bias_row_bcast("img_b", D, D)

    # adapter weights, resident for all three adapter calls (~10.5KB)
    ad1w = []
    for k in range(KD):
        wt = gct([P, DQ], f32r, f"ad1w{k}")
        dma(wt[:], dr["ad1_w"][k * P:(k + 1) * P, :].bitcast(f32r))
        ad1w.append(wt)
    ad2w = []
    for k, kp in enumerate([P, DQ - P]):
        wt = gct([kp, D], f32r, f"ad2w{k}")
        dma(wt[:], dr["ad2_w"][k * P:k * P + kp, :].bitcast(f32r))
        ad2w.append(wt)


    # ---- encoder branch (independent of the decoder stream) ----
    s_kv = Pool(tc, "st_kv", side="right")
    enc = Pool(tc, "ph_enc")
    xe = load_transposed(dr["x_enc"], NE, TT_NE, enc, "xe", 6, n_pad=NEP)
    he = layernorm(xe, NEP, CH_NE, b_ln3g, b_ln3b, enc, "he", 6, "3")
    encp = Pool(tc, "ph_encp", space="PSUM")
    k2T = proj_to_tiles(enc, encp, he, dr["img_w"], 0, D, CH_NE, NEP,
                        b_img, 0, s_kv, "k2", 6)
    v2ext = proj_tm_vext(enc, encp, he, dr["img_w"], D, TT_NE_KV, vb_img,
                         s_kv, "v2ext", 3)
    encp.release()
    enc.release()

    # ---- self-attention ----
    # overlapping stage lifetimes alternate allocation sides (LIFO per side)
    s_qk = Pool(tc, "st_qk", side="right")
    s_h = Pool(tc, "st_h")
    h1 = layernorm(x0, T, CH_T, b_ln1g, b_ln1b, s_h, "h", 6, "1")
    ph = Pool(tc, "ph_qkv")
    php = Pool(tc, "ph_qkvp", space="PSUM")
    qT = proj_to_tiles(ph, php, h1, dr["attn_w"], 0, D, CH_T, T,
                       b_attn, 0, s_qk, "qk", 12)
    kT = proj_to_tiles(ph, php, h1, dr["attn_w"], D, D, CH_T, T,
                       b_attn, KD, s_qk, "qk", 12)
    vext = proj_tm_vext(ph, php, h1, dr["attn_w"], 2 * D, TT_T, vb_attn,
                        s_qk, "vext", 8)
    php.release()
    ph.release()
    s_h.release()
    s_sa = Pool(tc, "st_sa")
    saT = attention(qT, kT, vext, TT_T, CH_T, True, s_sa, "sa", "1")
    s_qk.release()
    x1 = proj_adapter_resid(saT, x0, dr["attn_proj_w"], b_aproj, "1")
    s_sa.release()

    # ---- cross-attention ----
    s_q2 = Pool(tc, "st_q2", side="right")
    s_h2 = Pool(tc, "st_h2")
    hq = layernorm(x1, T, CH_T, b_ln1g, b_ln1b, s_h2, "h", 6, "q")
    ph2 = Pool(tc, "ph_q2")
    ph2p = Pool(tc, "ph_q2p", space="PSUM")
    q2T = proj_to_tiles(ph2, ph2p, hq, dr["cap_w"], 0, D, CH_T, T,
                        b_cap, 0, s_q2, "q2", 6)
    ph2p.release()
    ph2.release()
    s_h2.release()
    s_ca = Pool(tc, "st_ca")
    caT = attention(q2T, k2T, v2ext, TT_NE_KV, CH_T, False, s_ca, "ca", "2",
                    tail_mask_ki=2)
    s_q2.release()
    s_kv.release()
    x2 = proj_adapter_resid(caT, x1, dr["cross_proj_w"], b_cproj, "2")
    s_ca.release()

    # ---- MLP (fp8 + DoubleRow; H4 contracted in quarters) ----
    # The MLP's output only reaches the residual through the 0.02-scale
    # adapter bottleneck, so fp8 quantization here adds ~3e-4 output error
    # while halving PE time for the two big GEMMs.
    s_m = Pool(tc, "st_m", side="right")
    s_hm = Pool(tc, "st_hm")
    # hm grouped in one tile so DoubleRow can pair adjacent feature tiles
    hm_big = s_hm.t([P, KD, T], f8, "hmbig", 1)
    hm = [hm_big[:, k, :] for k in range(KD)]
    layernorm(x2, T, CH_T, b_ln2g, b_ln2b, s_hm, "h", 6, "2", out_aps=hm)
    m = [s_m.t([P, T], f32r, "m", 6) for _ in range(KD)]
    mp = Pool(tc, "ph_mlp")
    mpp = Pool(tc, "ph_mlpp", space="PSUM")
    macc = [mp.t([P, T], f32, "macc", 6) for _ in range(KD)]
    NQ = 4                       # quarters
    QH = H4 // NQ                # 768 cols per quarter
    NKH = QH // P                # 6 contraction tiles per quarter
    def fc_quarter(quarter):
        hc0 = quarter * QH
        gt_big = mp.t([P, NKH, T], f8, "gtbig", 2)
        CW = 512
        for c in range(0, QH, CW):
            cw = min(CW, QH - c)
            wts = []
            for j in range(KD // 2):
                wtmp = mp.t([P, 2, cw], f32, "fcwf", 3)
                dma(wtmp[:], dr["fc_w"][2 * j * P:(2 * j + 2) * P,
                                        hc0 + c:hc0 + c + cw]
                    .rearrange("(j p) m -> p j m", p=P))
                wt = mp.t([P, 2, cw], f8, "fcw", 3)
                V.tensor_copy(wt[:], wtmp[:])
                wts.append(wt)
            for (q0, qw) in CH_T:
                o = 0
                while o < cw:
                    nsz = min(P, cw - o)
                    nt = (c + o) // P
                    pq = mpp.t([nsz, qw], f32, "fcps", 4)
                    for j in range(KD // 2):
                        TE.matmul(pq[:], wts[j][:, :, o:o + nsz],
                                  hm_big[:, 2 * j:2 * j + 2, q0:q0 + qw],
                                  start=(j == 0), stop=(j == KD // 2 - 1),
                                  perf_mode=mybir.MatmulPerfMode.DoubleRow)
                    bsl = b_fc[:, (hc0 + nt * P) // P:(hc0 + nt * P) // P + 1]
                    S.activation(gt_big[:, nt, q0:q0 + qw], pq[:],
                                 AF.Gelu_apprx_tanh, bias=bsl)
                    o += nsz
        mpw = []
        for j in range(NKH // 2):
            wtmp = mp.t([P, 2, D], f32, "mpwf", 3)
            dma(wtmp[:], dr["mproj_w"][hc0 + 2 * j * P:hc0 + (2 * j + 2) * P, :]
                .rearrange("(j p) m -> p j m", p=P))
            wt = mp.t([P, 2, D], f8, "mpw", 6)
            V.tensor_copy(wt[:], wtmp[:])
            mpw.append(wt)
        return gt_big, mpw

    def mproj_quarter(quarter, gt_big, mpw):
        for (q0, qw) in CH_T:
            for nt in range(KD):
                pq = mpp.t([P, qw], f32, "mpps", 4)
                for j in range(NKH // 2):
                    TE.matmul(pq[:], mpw[j][:, :, nt * P:(nt + 1) * P],
                              gt_big[:, 2 * j:2 * j + 2, q0:q0 + qw],
                              start=(j == 0), stop=(j == NKH // 2 - 1),
                              perf_mode=mybir.MatmulPerfMode.DoubleRow)
                if quarter == 0:
                    V.tensor_copy(macc[nt][:, q0:q0 + qw], pq[:])
                elif quarter < NQ - 1:
                    V.tensor_add(macc[nt][:, q0:q0 + qw], pq[:],
                                 macc[nt][:, q0:q0 + qw])
                else:
                    V.scalar_tensor_tensor(m[nt][:, q0:q0 + qw], pq[:],
                                           b_mproj[:, nt:nt + 1],
                                           macc[nt][:, q0:q0 + qw],
                                           OP.add, OP.add)

    # software pipeline: quarter q's mproj is emitted after quarter q+1's fc
    prev = None
    for quarter in range(NQ):
        cur = fc_quarter(quarter)
        if prev is not None:
            mproj_quarter(quarter - 1, *prev)
        prev = cur
    mproj_quarter(NQ - 1, *prev)
    mpp.release()
    mp.release()
    s_hm.release()
    # output transpose/store fused into adapter3: each token chunk streams
    # out while the other chunk's adapter still computes
    op_ = Pool(tc, "ph_out")
    opp = Pool(tc, "ph_outp", space="PSUM")

    def store_qc(q0, qw):
        for (t0, tp) in TT_T:
            if t0 < q0 or t0 >= q0 + qw:
                continue
            ot = op_.t([P, D], f32, "ot", 4)
            for k in range(KD):
                pst = opp.t([P, P], f32r, "otps", 4)
                TE.transpose(pst[:], x2[k][:, t0:t0 + tp], ident[:])
                V.tensor_copy(ot[:, k * P:(k + 1) * P], pst[:])
            dma(dr["out"][t0:t0 + tp, :], ot[:])

    x3 = proj_adapter_resid(m, x2, None, None, "3", post_qc=store_qc)
    s_m.release()
    opp.release()
    op_.release()
    gr.release()
    gc.release()


def build():
    nc = bacc.Bacc("TRN2", target_bir_lowering=False, debug=False,
                   num_devices=8)
    dr = {}
    dr["x"] = nc.dram_tensor("x", (T, D), f32, kind="ExternalInput")
    dr["x_enc"] = nc.dram_tensor("x_enc", (NE, D), f32, kind="ExternalInput")
    for nm, shp in [
        ("ln1_g", (D,)), ("ln1_b", (D,)), ("ln2_g", (D,)), ("ln2_b", (D,)),
        ("ln3_g", (D,)), ("ln3_b", (D,)),
        ("attn_w", (D, 3 * D)), ("attn_bias", (3 * D,)),
        ("attn_proj_w", (D, D)), ("attn_proj_b", (D,)),
        ("img_w", (D, 2 * D)), ("img_b", (2 * D,)),
        ("cap_w", (D, D)), ("cap_b", (D,)),
        ("cross_proj_w", (D, D)), ("cross_proj_b", (D,)),
        ("ad1_w", (D, DQ)), ("ad1_b", (DQ,)),
        ("ad2_w", (DQ, D)), ("ad2_b", (D,)),
        ("fc_w", (D, H4)), ("fc_b", (H4,)),
        ("mproj_w", (H4, D)), ("mproj_b", (D,)),
    ]:
        dr[nm] = nc.dram_tensor(nm, shp, f32, kind="ExternalInput")
    dr["out"] = nc.dram_tensor("out", (T, D), f32, kind="ExternalOutput")

    with tile.TileContext(nc) as tc:
        emit(nc, tc, dr)
    nc.compile()
    return nc


_NC = None


def kernel(**inputs):
    global _NC
    if _NC is None:
        _NC = build()
    nc = _NC
    B = inputs["x"].shape[0]
    in_maps = []
    for b in range(B):
        m = {}
        for n, v in inputs.items():
            v = np.asarray(v, dtype=np.float32)
            if n in ("x", "x_enc"):
                m[n] = np.ascontiguousarray(v[b])
            else:
                m[n] = np.ascontiguousarray(v)
        in_maps.append(m)
    res = bass_utils.run_bass_kernel_spmd(nc, in_maps, list(range(B)))
    x_out = np.stack([res.results[b]["out"] for b in range(B)], axis=0)
    # the block returns (x, x_enc); x_enc passes through unchanged
    x_enc = np.asarray(inputs["x_enc"], dtype=np.float32)
    return x_out, x_enc
